# revision 7
# baseline (speedup 1.0000x reference)
"""Tensor-parallel GQA attention forward for Trainium2 (8 NeuronCores).

Sharding: tensor-parallel over heads.  Each core owns 4 q-heads and 1
kv-head (wq/wk/wv output-dim shard, wo input-dim shard), processes the
full 4096-token stream, and a ReduceScatter combines the o-proj partial
sums so core c ends with output token rows [c*512, (c+1)*512).

Device pipeline per core (all matmuls float32r, ~1.6e-4 relative):
  1. QKV projections from host-pretransposed x^T, fused RoPE (even/odd
     dims pre-separated by a host permutation of wq/wk rows so the
     rotation acts on contiguous 64-wide halves), PE transposes to get
     Q^T (spilled to DRAM) and K^T (SBUF-resident).  V stays natural.
  2. Streaming attention per (batch, head, 512-token piece): S^T tile =
     K^T-chunk.T @ Q^T-piece, exp on ScalarE (no max subtraction - the
     unmasked scores are O(10)), PV and ones-row sums accumulate in
     PSUM over the 16 s-chunks, reciprocal + PE-broadcast normalize.
  3. o-proj from SBUF-resident ctx^T with host-pretransposed wo shard.
  4. ReduceScatter over the 8 cores; host concatenates the slices.
"""
import math
import numpy as np

import concourse.bacc as bacc
import concourse.mybir as mybir
import concourse.tile as tile
from concourse import bass_utils

F32R = mybir.dt.float32r
F32 = mybir.dt.float32
AF = mybir.ActivationFunctionType

N_CORES = 8
B, T, DIM = 2, 2048, 4096
N_HEADS, N_KV_HEADS, HD = 32, 8, 128
HL = N_HEADS // N_CORES            # 4 q heads per core
TOK = B * T                        # 4096
KCH = DIM // 128                   # 32 contraction chunks
NTT = TOK // 128                   # 32 token tiles
QW = HL * HD                       # 512
PROJW = QW + 2 * HD                # 768 (q | k | v)
SCALE = 1.0 / math.sqrt(HD)
NSC = T // 128                     # 16 s-chunks per batch
NTP = T // 512                     # 4 t-pieces per batch
OSL = TOK // N_CORES               # 512 output rows per core

_CACHE = {}


def _build(collective=True):
    nc = bacc.Bacc("TRN2", target_bir_lowering=False, debug=False,
                   num_devices=N_CORES if collective else 1)
    xT = nc.dram_tensor("xT", [DIM, TOK], F32R, kind="ExternalInput")
    wqkvT = nc.dram_tensor("wqkvT", [DIM, PROJW], F32R, kind="ExternalInput")
    woT = nc.dram_tensor("woT", [QW, DIM], F32R, kind="ExternalInput")
    cosn = nc.dram_tensor("cosn", [TOK, 4 * 64], F32R, kind="ExternalInput")
    sinn = nc.dram_tensor("sinn", [TOK, 4 * 64], F32R, kind="ExternalInput")
    ones_col = nc.dram_tensor("ones_col", [128, 1], F32R, kind="ExternalInput")
    ones_row = nc.dram_tensor("ones_row", [1, 128], F32R, kind="ExternalInput")
    ident = nc.dram_tensor("ident", [128, 128], F32R, kind="ExternalInput")
    out_sl = nc.dram_tensor("out_sl", [OSL, DIM], F32, kind="ExternalOutput")

    with tile.TileContext(nc) as tc:
        with (
            nc.allow_low_precision(reason="float32r intermediates are f32 bits"),
            tc.tile_pool(name="res", bufs=1) as res,
            tc.tile_pool(name="dram", bufs=1, space="DRAM") as dram,
        ):
            kT_all = res.tile([128, TOK], F32R, tag="kT")
            v_all = res.tile([128, TOK], F32R, tag="v")
            oc_t = res.tile([128, 1], F32R, tag="oc")
            or_t = res.tile([1, 128], F32R, tag="or")
            id_t = res.tile([128, 128], F32R, tag="id")
            nc.sync.dma_start(out=oc_t[:], in_=ones_col[:])
            nc.sync.dma_start(out=or_t[:], in_=ones_row[:])
            nc.sync.dma_start(out=id_t[:], in_=ident[:])

            qT_d = dram.tile([QW, TOK], F32R)
            rs_in = dram.tile([TOK, DIM], F32)
            rs_out = dram.tile([OSL, DIM], F32)

            # ------------- Phase 1: projections + RoPE + transposes ------
            with (
                tc.tile_pool(name="p1w", bufs=1) as p1w,
                tc.tile_pool(name="p1s", bufs=2) as p1s,
                tc.tile_pool(name="ps1", bufs=2, space="PSUM") as ps1,
            ):
                w_t = p1w.tile([128, KCH * PROJW], F32R, tag="w")
                nc.sync.dma_start(
                    out=w_t[:].rearrange("p (kc q) -> p kc q", q=PROJW),
                    in_=wqkvT[:].rearrange("(kc p) q -> p kc q", p=128),
                )
                for tt in range(NTT):
                    xt = p1s.tile([128, KCH * 128], F32R, tag="xt")
                    nc.sync.dma_start(
                        out=xt[:].rearrange("p (kc t) -> p kc t", t=128),
                        in_=xT[:, tt * 128:(tt + 1) * 128].rearrange(
                            "(kc p) t -> p kc t", p=128),
                    )
                    q_ps = ps1.tile([128, QW], F32, tag="q")
                    kv_ps = ps1.tile([128, 2 * HD], F32, tag="kv")
                    for kc in range(KCH):
                        nc.tensor.matmul(
                            q_ps[:], xt[:, kc * 128:(kc + 1) * 128],
                            w_t[:, kc * PROJW: kc * PROJW + QW],
                            start=(kc == 0), stop=(kc == KCH - 1),
                        )
                        nc.tensor.matmul(
                            kv_ps[:], xt[:, kc * 128:(kc + 1) * 128],
                            w_t[:, kc * PROJW + QW: (kc + 1) * PROJW],
                            start=(kc == 0), stop=(kc == KCH - 1),
                        )
                    # RoPE (even/odd pre-separated into 64-wide halves)
                    ct = p1s.tile([128, 4 * 64], F32R, tag="cos")
                    st = p1s.tile([128, 4 * 64], F32R, tag="sin")
                    nc.sync.dma_start(out=ct[:], in_=cosn[tt * 128:(tt + 1) * 128, :])
                    nc.sync.dma_start(out=st[:], in_=sinn[tt * 128:(tt + 1) * 128, :])
                    rot = p1s.tile([128, QW + HD], F32R, tag="rot")
                    t1 = p1s.tile([128, 4 * 64], F32R, tag="t1")

                    qv = q_ps[:].rearrange("p (u hf) -> p u hf", hf=128)
                    qe, qo = qv[:, :, 0:64], qv[:, :, 64:128]
                    rv = rot[:, 0:QW].rearrange("p (u hf) -> p u hf", hf=128)
                    re, ro = rv[:, :, 0:64], rv[:, :, 64:128]
                    cv = ct[:].rearrange("p (u f) -> p u f", f=64)
                    sv = st[:].rearrange("p (u f) -> p u f", f=64)
                    tv = t1[:].rearrange("p (u f) -> p u f", f=64)
                    nc.vector.tensor_mul(re, qe, cv)
                    nc.vector.tensor_mul(tv, qo, sv)
                    nc.vector.tensor_sub(re, re, tv)
                    nc.vector.tensor_mul(ro, qe, sv)
                    nc.vector.tensor_mul(tv, qo, cv)
                    nc.vector.tensor_add(ro, ro, tv)
                    # k rope
                    ke, ko = kv_ps[:, 0:64], kv_ps[:, 64:128]
                    kre, kro = rot[:, QW:QW + 64], rot[:, QW + 64:QW + 128]
                    c1, s1, t1s = ct[:, 0:64], st[:, 0:64], t1[:, 0:64]
                    nc.vector.tensor_mul(kre, ke, c1)
                    nc.vector.tensor_mul(t1s, ko, s1)
                    nc.vector.tensor_sub(kre, kre, t1s)
                    nc.vector.tensor_mul(kro, ke, s1)
                    nc.vector.tensor_mul(t1s, ko, c1)
                    nc.vector.tensor_add(kro, kro, t1s)
                    # v copy (natural layout, chunk tt)
                    nc.scalar.copy(v_all[:, tt * 128:(tt + 1) * 128],
                                   kv_ps[:, 128:256])
                    # transposes: 4 q heads -> DRAM, 1 k -> resident K^T
                    for u in range(HL + 1):
                        tp_ps = ps1.tile([128, 128], F32R, tag="tp")
                        nc.tensor.transpose(
                            tp_ps[:], rot[:, u * 128:(u + 1) * 128], id_t[:])
                        if u < HL:
                            stg = p1s.tile([128, 128], F32R, tag="qstage")
                            nc.scalar.copy(stg[:], tp_ps[:])
                            nc.sync.dma_start(
                                out=qT_d[u * 128:(u + 1) * 128,
                                         tt * 128:(tt + 1) * 128],
                                in_=stg[:],
                            )
                        else:
                            nc.scalar.copy(
                                kT_all[:, tt * 128:(tt + 1) * 128], tp_ps[:])

            # ------------- Phase 2: attention; Phase 3: o-proj -----------
            with (
                tc.tile_pool(name="p2w", bufs=1) as p2w,
                tc.tile_pool(name="ctxp", bufs=2) as ctxp,
            ):
                wo_t = p2w.tile([128, HL * DIM], F32R, tag="wo")
                nc.sync.dma_start(
                    out=wo_t[:].rearrange("p (cc d) -> p cc d", d=DIM),
                    in_=woT[:].rearrange("(cc p) d -> p cc d", p=128),
                )
                ctx_tiles = []
                with (
                    tc.tile_pool(name="p2s", bufs=3) as p2s,
                    tc.tile_pool(name="ps2", bufs=2, space="PSUM") as ps2,
                ):
                  for b in range(B):
                    ctx_all = ctxp.tile([128, HL * T], F32R, tag="ctx")
                    ctx_tiles.append(ctx_all)
                    for h in range(HL):
                        for tp in range(NTP):
                            qt = p2s.tile([128, 512], F32R, tag="qt")
                            nc.sync.dma_start(
                                out=qt[:],
                                in_=qT_d[h * 128:(h + 1) * 128,
                                         b * T + tp * 512: b * T + (tp + 1) * 512],
                            )
                            ctx_ps = ps2.tile([128, 512], F32, tag="ctx")
                            sums_ps = ps2.tile([1, 512], F32, tag="sums")
                            for sc in range(NSC):
                                g = (b * NSC + sc) * 128
                                s_ps = ps2.tile([128, 512], F32, tag="s")
                                nc.tensor.matmul(
                                    s_ps[:], kT_all[:, g:g + 128], qt[:],
                                    start=True, stop=True,
                                )
                                p_t = p2s.tile([128, 512], F32R, tag="p")
                                nc.scalar.activation(
                                    p_t[:], s_ps[:], AF.Exp, scale=SCALE)
                                nc.tensor.matmul(
                                    ctx_ps[:], v_all[:, g:g + 128], p_t[:],
                                    start=(sc == 0), stop=(sc == NSC - 1),
                                )
                                nc.tensor.matmul(
                                    sums_ps[:], oc_t[:], p_t[:],
                                    start=(sc == 0), stop=(sc == NSC - 1),
                                )
                            recip = p2s.tile([1, 512], F32R, tag="recip")
                            nc.vector.reciprocal(recip[:], sums_ps[:])
                            bc_ps = ps2.tile([128, 512], F32, tag="s")
                            nc.tensor.matmul(bc_ps[:], or_t[:], recip[:],
                                             start=True, stop=True)
                            cslice = ctx_all[:, h * T + tp * 512: h * T + (tp + 1) * 512]
                            nc.vector.tensor_copy(cslice, ctx_ps[:])
                            nc.vector.tensor_mul(cslice, cslice, bc_ps[:])

                # phase 3: o-proj (own pool scope; interleaving its PSUM
                # pool with phase 2's hangs the device)
                with (
                    tc.tile_pool(name="p3s", bufs=3) as p3s,
                    tc.tile_pool(name="ps3", bufs=2, space="PSUM") as ps3,
                ):
                  for b in range(B):
                    ctx_all = ctx_tiles[b]
                    for tt16 in range(T // 128):
                        for dq in range(4):
                            o_ps = ps3.tile([128, 1024], F32, tag="o")
                            for cc in range(HL):
                                lhs = ctx_all[:, cc * T + tt16 * 128:
                                              cc * T + (tt16 + 1) * 128]
                                for dp in range(2):
                                    nc.tensor.matmul(
                                        o_ps[:, dp * 512:(dp + 1) * 512],
                                        lhs,
                                        wo_t[:, cc * DIM + dq * 1024 + dp * 512:
                                             cc * DIM + dq * 1024 + (dp + 1) * 512],
                                        start=(cc == 0), stop=(cc == HL - 1),
                                    )
                            ost = p3s.tile([128, 1024], F32, tag="ost")
                            nc.vector.tensor_copy(ost[:], o_ps[:])
                            nc.sync.dma_start(
                                out=rs_in[b * T + tt16 * 128:
                                          b * T + (tt16 + 1) * 128,
                                          dq * 1024:(dq + 1) * 1024],
                                in_=ost[:],
                            )

            if collective:
                nc.gpsimd.collective_compute(
                    "ReduceScatter",
                    mybir.AluOpType.add,
                    replica_groups=[list(range(N_CORES))],
                    ins=[rs_in[:].opt()],
                    outs=[rs_out[:].opt()],
                )
                rs_src = rs_out
            else:
                rs_src = rs_in  # sim: core 0's partial, first OSL rows
            with tc.tile_pool(name="outp", bufs=2) as outp:
                for j in range(OSL // 128):
                    ot = outp.tile([128, DIM], F32, tag="ot")
                    nc.sync.dma_start(out=ot[:], in_=rs_src[j * 128:(j + 1) * 128, :])
                    nc.sync.dma_start(out=out_sl[j * 128:(j + 1) * 128, :], in_=ot[:])
    nc.compile()
    return nc


def _rope_permutation():
    """Per-head permutation putting even dims first, odd dims second."""
    perm = np.empty(HD, dtype=np.int64)
    perm[:HD // 2] = np.arange(0, HD, 2)
    perm[HD // 2:] = np.arange(1, HD, 2)
    return perm


def _prep_inputs(x, wq, wk, wv, wo, freqs_cos, freqs_sin):
    x2d = np.ascontiguousarray(np.asarray(x, dtype=np.float32).reshape(TOK, DIM))
    xT = np.ascontiguousarray(x2d.T)
    wq = np.asarray(wq, dtype=np.float32)
    wk = np.asarray(wk, dtype=np.float32)
    wv = np.asarray(wv, dtype=np.float32)
    wo = np.asarray(wo, dtype=np.float32)
    fc = np.asarray(freqs_cos, dtype=np.float32)
    fs = np.asarray(freqs_sin, dtype=np.float32)

    perm = _rope_permutation()
    cosn = np.ascontiguousarray(np.tile(np.concatenate([fc, fc], axis=0), (1, 4)))
    sinn = np.ascontiguousarray(np.tile(np.concatenate([fs, fs], axis=0), (1, 4)))
    ones_col = np.ones((128, 1), np.float32)
    ones_row = np.ones((1, 128), np.float32)
    ident = np.eye(128, dtype=np.float32)

    in_maps = []
    for c in range(N_CORES):
        # reference GQA (torch-style .repeat / jnp.tile): q-head g attends
        # kv-head g % 8, so core c owns q-heads {c, c+8, c+16, c+24} and
        # kv-head c.
        heads = [c + N_KV_HEADS * u for u in range(HL)]
        wq_c = wq.reshape(N_HEADS, HD, DIM)[heads][:, perm, :].reshape(QW, DIM)
        wk_c = wk[c * HD:(c + 1) * HD, :][perm, :]
        wv_c = wv[c * HD:(c + 1) * HD, :]
        wqkvT = np.ascontiguousarray(
            np.concatenate([wq_c, wk_c, wv_c], axis=0).T)
        wo_c = wo.reshape(DIM, N_HEADS, HD)[:, heads, :].reshape(DIM, QW)
        woT = np.ascontiguousarray(wo_c.T)
        in_maps.append({
            "xT": xT, "wqkvT": wqkvT, "woT": woT,
            "cosn": cosn, "sinn": sinn,
            "ones_col": ones_col, "ones_row": ones_row, "ident": ident,
        })
    return in_maps


def kernel(x, wq, wk, wv, wo, freqs_cos, freqs_sin,
           cache_k=None, cache_v=None, mask=None, start_pos=0, **_):
    assert int(start_pos) == 0, "kernel is specialized for start_pos=0"
    if "nc" not in _CACHE:
        _CACHE["nc"] = _build()
    nc = _CACHE["nc"]
    in_maps = _prep_inputs(x, wq, wk, wv, wo, freqs_cos, freqs_sin)
    res = bass_utils.run_bass_kernel_spmd(
        nc, in_maps, core_ids=list(range(N_CORES)))
    out = np.concatenate(
        [res.results[c]["out_sl"] for c in range(N_CORES)], axis=0)
    return out.reshape(B, T, DIM)
